# revision 15
# baseline (speedup 1.0000x reference)
"""DGE block kernel for 8 Trainium2 NeuronCores.

Sharding: data-parallel over (b, n-half): core k handles batch b=k//2,
n-half h=k%2. Each core receives x[b] with its own n-half rolled to the
front, computes the full adjacency A[b] (needs mean over all N), and
produces H_out[b, :, half, :].

Per-core device pipeline (all fp32 / fp32r):
  1. HcT via indicator-matmul accumulation over all N      (PE)
  2. Q/K proj, dA=tanh(QK^T/8), symmetrize, degree-norm    (PE+ACT+DVE, tiny)
  3. main loop over 32 batches of 512 tokens:
       (A+I) @ x  -> PSUM            (PE, blockdiag-packed 2x64)
       LN1 stats+apply               (DVE bn_stats + ACT)
       PE transpose -> hnT [d, tok]  (PE + DVE copy)
       FFN1: W1'^T blocks @ hnT -> z1T [dff, tok] (PE)
       gelu(+b1) ACT -> gT           (ACT, bias per-partition)
       FFN2: gT^T @ W2 -> out2 [tok, d] (PE)
       r2 = hn + out2, LN2 stats+apply -> H_out (DVE + ACT)
"""

import sys

import numpy as np

B, C, N, D, DK, DFF = 4, 64, 512, 256, 64, 1024
NH = N // 2  # per-core n-half
EPS_LN, EPS_DEG = 1e-5, 1e-6
ATT_SCALE = 8.0  # sqrt(DK) * TAU
NCORES = 8
NBATCH = 32  # batches of 8 n-values (512 tokens) per core

_CACHE = {}


def _ensure_path():
    try:
        import concourse  # noqa: F401
    except ImportError:
        sys.path.insert(0, "/opt/trn_rl_repo")


def _build_program(flags):
    """Build the Bass/Tile program. flags = (g1_trivial, b2_zero, g2_trivial)."""
    from contextlib import ExitStack

    import concourse.bacc as bacc
    import concourse.bass as bass
    import concourse.tile as tile
    from concourse import mybir
    from concourse.masks import make_identity

    g1_trivial, b2_zero, g2_trivial = flags

    fp32 = mybir.dt.float32
    fp32r = mybir.dt.float32r
    F = mybir.ActivationFunctionType
    OP = mybir.AluOpType

    nc = bacc.Bacc("TRN2", target_bir_lowering=False, debug=False,
                   enable_asserts=False, num_devices=NCORES)

    # matmul-fed inputs are declared float32r (same 4-byte container; the
    # PE rounds internally) so the whole operand chain carries the dtype
    # the BIR verifier requires for fp32r matmuls.
    xs = nc.dram_tensor("xs", [C, N, D], fp32r, kind="ExternalInput").ap()
    a_st = nc.dram_tensor("a_st", [C, C], fp32, kind="ExternalInput").ap()
    wqk = nc.dram_tensor("wqk", [D, 2 * DK], fp32r, kind="ExternalInput").ap()
    w1 = nc.dram_tensor("w1", [D, DFF], fp32r, kind="ExternalInput").ap()
    b1 = nc.dram_tensor("b1", [DFF], fp32, kind="ExternalInput").ap()
    w2 = nc.dram_tensor("w2", [DFF, D], fp32r, kind="ExternalInput").ap()
    b2 = None if b2_zero else nc.dram_tensor("b2", [D], fp32, kind="ExternalInput").ap()
    g2be2 = None
    if not g2_trivial:
        g2be2 = nc.dram_tensor("g2be2", [2, D], fp32, kind="ExternalInput").ap()
    g1be1 = None
    if not g1_trivial:
        g1be1 = nc.dram_tensor("g1be1", [2, D], fp32, kind="ExternalInput").ap()
    yh = nc.dram_tensor("yh", [C, NH, D], fp32, kind="ExternalOutput").ap()
    a_out = nc.dram_tensor("a_out", [C, C], fp32, kind="ExternalOutput").ap()

    def r(ap):
        return ap if ap.dtype == fp32r else ap.bitcast(fp32r)

    with tile.TileContext(nc) as tc:
        with ExitStack() as ctx:
            consts = ctx.enter_context(tc.tile_pool(name="consts", bufs=1))

            id128 = consts.tile([128, 128], fp32)
            make_identity(nc, id128)
            eye64 = id128[0:64, 0:64]

            # off-diagonal mask (1 everywhere, 0 on diag)
            mask64 = consts.tile([64, 64], fp32)
            nc.gpsimd.memset(mask64, 1.0)
            nc.gpsimd.affine_select(
                out=mask64, in_=mask64,
                compare_op=OP.not_equal, fill=0.0,
                base=0, pattern=[[-1, 64]], channel_multiplier=1,
            )

            # sliding indicator for Hc: ind[:, 63] = 1/N, else 0.
            # lhsT for channel c = ind[:, 63-c : 127-c]  (col c == 1/N)
            ind_f = consts.tile([128, 127], fp32)
            nc.vector.memset(ind_f, 0.0)
            nc.vector.memset(ind_f[:, 63:64], 1.0 / N)
            ind = consts.tile([128, 127], fp32r)
            nc.vector.tensor_copy(ind, ind_f)

            eps1 = consts.tile([128, 1], fp32)
            nc.vector.memset(eps1, EPS_LN)

            w1sb = consts.tile([128, 2, DFF], fp32r)
            nc.sync.dma_start(out=w1sb, in_=w1.rearrange("(k p) f -> p k f", k=2))
            w2sb = consts.tile([128, 8, D], fp32r)
            nc.sync.dma_start(out=w2sb, in_=w2.rearrange("(k p) f -> p k f", k=8))
            wqksb = consts.tile([128, 2, 2 * DK], fp32r)
            nc.sync.dma_start(out=wqksb, in_=wqk.rearrange("(k p) f -> p k f", k=2))
            b1sb = consts.tile([128, 8], fp32)
            nc.sync.dma_start(out=b1sb, in_=b1.rearrange("(k p) -> p k", k=8))
            assb = consts.tile([64, 64], fp32)
            nc.sync.dma_start(out=assb, in_=a_st)
            bd = consts.tile([128, 128], fp32r)

            b2bc = None
            if b2 is not None:
                b2bc = consts.tile([128, D], fp32)
                nc.sync.dma_start(
                    out=b2bc,
                    in_=bass.AP(tensor=b2.tensor, offset=b2.offset,
                                ap=[[0, 128], [1, D]]))
            g2bc = be2bc = None
            if g2be2 is not None:
                g2bc = consts.tile([128, D], fp32)
                be2bc = consts.tile([128, D], fp32)
                nc.sync.dma_start(
                    out=g2bc,
                    in_=bass.AP(tensor=g2be2.tensor, offset=g2be2.offset,
                                ap=[[0, 128], [1, D]]))
                nc.sync.dma_start(
                    out=be2bc,
                    in_=bass.AP(tensor=g2be2.tensor, offset=g2be2.offset + D,
                                ap=[[0, 128], [1, D]]))
            g1bc = be1bc = None
            if g1be1 is not None:
                g1bc = consts.tile([128, D], fp32)
                be1bc = consts.tile([128, D], fp32)
                nc.sync.dma_start(
                    out=g1bc,
                    in_=bass.AP(tensor=g1be1.tensor, offset=g1be1.offset,
                                ap=[[0, 128], [1, D]]))
                nc.sync.dma_start(
                    out=be1bc,
                    in_=bass.AP(tensor=g1be1.tensor, offset=g1be1.offset + D,
                                ap=[[0, 128], [1, D]]))

            # ---------------- adjacency phase ----------------
            with ExitStack() as adj_ctx:
                adjp = adj_ctx.enter_context(tc.tile_pool(name="adjp", bufs=2))
                hcxp = adj_ctx.enter_context(tc.tile_pool(name="hcxp", bufs=4))
                adjps = adj_ctx.enter_context(
                    tc.tile_pool(name="adjps", bufs=1, space="PSUM"))

                # Hc accumulation: hc_ps[c, (j, d)] over all (c, n)-tiles
                hc_ps = adjps.tile([64, 2, D], fp32)
                for c in range(C):
                    for jj in range(2):
                        hcx = hcxp.tile([128, 2, D], fp32r, tag="hcx")
                        nc.sync.dma_start(
                            out=hcx,
                            in_=xs[c, jj * 256:(jj + 1) * 256, :]
                            .rearrange("(j p) d -> p j d", j=2))
                        nc.tensor.matmul(
                            hc_ps, r(ind[:, 63 - c:127 - c]), r(hcx),
                            start=(c == 0 and jj == 0),
                            stop=(c == C - 1 and jj == 1))

                hc_sb = adjp.tile([64, D], fp32)
                nc.vector.tensor_copy(hc_sb, hc_ps[:, 0, :])
                nc.vector.tensor_add(hc_sb, hc_sb, hc_ps[:, 1, :])

                # HcT [d(2x128), c]
                hcT = adjp.tile([128, 2, 64], fp32r)
                for j in range(2):
                    tp = adjps.tile([128, 64], fp32, tag="tp")
                    nc.tensor.transpose(tp, hc_sb[:, j * 128:(j + 1) * 128], eye64)
                    nc.vector.tensor_copy(hcT[:, j, :], tp)

                # QT, KT [a=64, c=64]
                qt_ps = adjps.tile([64, 64], fp32)
                kt_ps = adjps.tile([64, 64], fp32)
                for ch in range(2):
                    nc.tensor.matmul(qt_ps, r(wqksb[:, ch, 0:64]), r(hcT[:, ch, :]),
                                     start=(ch == 0), stop=(ch == 1))
                for ch in range(2):
                    nc.tensor.matmul(kt_ps, r(wqksb[:, ch, 64:128]), r(hcT[:, ch, :]),
                                     start=(ch == 0), stop=(ch == 1))
                qt_sb = adjp.tile([64, 64], fp32r)
                nc.vector.tensor_copy(qt_sb, qt_ps)
                kt_sb = adjp.tile([64, 64], fp32r)
                nc.vector.tensor_copy(kt_sb, kt_ps)

                # S = Q @ K^T ; dA = tanh(S/8)
                s_ps = adjps.tile([64, 64], fp32)
                nc.tensor.matmul(s_ps, r(qt_sb), r(kt_sb), start=True, stop=True)
                da_sb = adjp.tile([64, 64], fp32)
                nc.scalar.activation(da_sb, s_ps, F.Tanh, scale=1.0 / ATT_SCALE)

                # symmetrize: afull = a_static + 0.5*(dA + dA^T); zero diag
                dat_ps = adjps.tile([64, 64], fp32)
                nc.tensor.transpose(dat_ps, da_sb, eye64)
                t1 = adjp.tile([64, 64], fp32)
                nc.vector.tensor_add(t1, da_sb, dat_ps)
                afull = adjp.tile([64, 64], fp32)
                nc.vector.scalar_tensor_tensor(afull, t1, 0.5, assb,
                                               op0=OP.mult, op1=OP.add)
                nc.vector.tensor_mul(afull, afull, mask64)

                # degree normalize: An = diag(dis) A diag(dis)
                deg = adjp.tile([64, 1], fp32)
                nc.vector.tensor_reduce(deg, afull, axis=mybir.AxisListType.X,
                                        op=OP.add)
                nc.vector.tensor_scalar_max(deg, deg, EPS_DEG)
                sq = adjp.tile([64, 1], fp32)
                nc.scalar.activation(sq, deg, F.Sqrt)
                dis = adjp.tile([64, 1], fp32)
                nc.vector.reciprocal(dis, sq)
                m1 = adjp.tile([64, 64], fp32)
                nc.vector.tensor_scalar_mul(m1, afull, dis)
                m1t_ps = adjps.tile([64, 64], fp32)
                nc.tensor.transpose(m1t_ps, m1, eye64)
                an_sb = adjp.tile([64, 64], fp32)
                nc.vector.tensor_scalar_mul(an_sb, m1t_ps, dis)
                nc.sync.dma_start(out=a_out, in_=an_sb)

                # blockdiag(A+I, A+I) for the propagate matmul
                at_sb = adjp.tile([64, 64], fp32)
                nc.vector.tensor_add(at_sb, an_sb, eye64)
                bd_f = adjp.tile([128, 128], fp32)
                nc.vector.memset(bd_f, 0.0)
                nc.sync.dma_start(out=bd_f[0:64, 0:64], in_=at_sb)
                nc.sync.dma_start(out=bd_f[64:128, 64:128], in_=at_sb)
                nc.vector.tensor_copy(bd, bd_f)

            # ---------------- main loop ----------------
            xbp = ctx.enter_context(tc.tile_pool(name="xbp", bufs=3))
            hnp = ctx.enter_context(tc.tile_pool(name="hnp", bufs=2))
            hntp = ctx.enter_context(tc.tile_pool(name="hntp", bufs=2))
            gtp = ctx.enter_context(tc.tile_pool(name="gtp", bufs=2))
            houtp = ctx.enter_context(tc.tile_pool(name="houtp", bufs=3))
            r2p = ctx.enter_context(tc.tile_pool(name="r2p", bufs=3))
            statp = ctx.enter_context(tc.tile_pool(name="statp", bufs=8))
            psap = ctx.enter_context(tc.tile_pool(name="psap", bufs=2, space="PSUM"))
            tpp = ctx.enter_context(tc.tile_pool(name="tpp", bufs=1, space="PSUM"))
            z1p = ctx.enter_context(tc.tile_pool(name="z1p", bufs=2, space="PSUM"))
            o2p = ctx.enter_context(tc.tile_pool(name="o2p", bufs=2, space="PSUM"))

            def ln_stats(src):
                """Return (inv, nbias) per-partition scalars for LN over free dim."""
                st = statp.tile([128, 6], fp32, tag="st")
                nc.vector.bn_stats(st, src)
                mv = statp.tile([128, 2], fp32, tag="mv")
                nc.vector.bn_aggr(mv, st)
                sd = statp.tile([128, 1], fp32, tag="sd")
                nc.scalar.activation(sd, mv[:, 1:2], F.Sqrt, bias=eps1)
                inv = statp.tile([128, 1], fp32, tag="inv")
                nc.vector.reciprocal(inv, sd)
                nb = statp.tile([128, 1], fp32, tag="nb")
                nc.vector.tensor_scalar(nb, mv[:, 0:1], inv, -1.0,
                                        op0=OP.mult, op1=OP.mult)
                return inv, nb

            def bc_view(base, n0, nstride, cstride):
                # [(blk,2), (c,64), (t,4), (d,256)] view with token
                # partition order p = blk*64 + c, n = n0 + blk*4 + t
                return bass.AP(
                    tensor=base.tensor, offset=base.offset + n0 * nstride,
                    ap=[[4 * nstride, 2], [cstride, C],
                        [nstride, 4], [1, D]])

            for s in range(NBATCH):
                n0 = s * 8
                xb = xbp.tile([128, 4, D], fp32r, tag="xb")
                nc.sync.dma_start(out=xb, in_=bc_view(xs, n0, D, N * D))

                hn = hnp.tile([128, 4, D], fp32, tag="hn")
                hnT = hntp.tile([128, 2, 512], fp32r, tag="hnT")

                # propagate + LN1
                for t2 in range(2):
                    psa = psap.tile([128, 2, D], fp32, tag="psa")
                    for u in range(2):
                        t = t2 * 2 + u
                        nc.tensor.matmul(psa[:, u, :], r(bd), r(xb[:, t, :]),
                                         start=True, stop=True)
                    for u in range(2):
                        t = t2 * 2 + u
                        inv, nb = ln_stats(psa[:, u, :])
                        nc.scalar.activation(hn[:, t, :], psa[:, u, :],
                                             F.Identity, bias=nb, scale=inv)

                # transpose hn -> hnT [d(2x128), tok=512]
                for t in range(4):
                    tp = tpp.tile([128, 2, 128], fp32, tag="tp")
                    for dch in range(2):
                        nc.tensor.transpose(
                            tp[:, dch, :], hn[:, t, dch * 128:(dch + 1) * 128],
                            id128)
                    nc.vector.tensor_copy(hnT[:, :, t * 128:(t + 1) * 128], tp)

                # FFN
                gT = gtp.tile([128, 8, 512], fp32r, tag="gT")
                o2a = o2p.tile([128, 2, D], fp32, tag="o2a", bufs=1)
                o2b = o2p.tile([128, 2, D], fp32, tag="o2b", bufs=1)
                o2 = [o2a, o2b]
                for ff in range(8):
                    z1 = z1p.tile([128, 512], fp32, tag="z1")
                    nc.tensor.matmul(z1, r(w1sb[:, 0, ff * 128:(ff + 1) * 128]),
                                     r(hnT[:, 0, :]), start=True, stop=False)
                    nc.tensor.matmul(z1, r(w1sb[:, 1, ff * 128:(ff + 1) * 128]),
                                     r(hnT[:, 1, :]), start=False, stop=True)
                    nc.scalar.activation(gT[:, ff, :], z1, F.Gelu,
                                         bias=b1sb[:, ff:ff + 1])
                # FFN2: per PSUM bank, groups must be sequential, so loop
                # t outer / ff inner once all gT chunks are materialized.
                for t in range(4):
                    for ff in range(8):
                        nc.tensor.matmul(
                            o2[t // 2][:, t % 2, :],
                            r(gT[:, ff, t * 128:(t + 1) * 128]),
                            r(w2sb[:, ff, :]),
                            start=(ff == 0), stop=(ff == 7))

                # residual2 + LN2
                houtb = houtp.tile([128, 4, D], fp32, tag="houtb")
                for t in range(4):
                    r2 = r2p.tile([128, D], fp32, tag="r2")
                    if g1_trivial:
                        nc.vector.scalar_tensor_tensor(
                            r2, o2[t // 2][:, t % 2, :], 1.0, hn[:, t, :],
                            op0=OP.mult, op1=OP.add)
                    else:
                        h1t = r2p.tile([128, D], fp32, tag="h1t")
                        nc.vector.tensor_mul(h1t, hn[:, t, :], g1bc)
                        nc.vector.tensor_add(h1t, h1t, be1bc)
                        nc.vector.tensor_add(r2, o2[t // 2][:, t % 2, :], h1t)
                    if b2bc is not None:
                        nc.vector.tensor_add(r2, r2, b2bc)
                    inv2, nb2 = ln_stats(r2)
                    if g2_trivial:
                        nc.scalar.activation(houtb[:, t, :], r2, F.Identity,
                                             bias=nb2, scale=inv2)
                    else:
                        nc.scalar.activation(houtb[:, t, :], r2, F.Identity,
                                             bias=nb2, scale=inv2)
                        nc.vector.tensor_mul(houtb[:, t, :], houtb[:, t, :], g2bc)
                        nc.vector.tensor_add(houtb[:, t, :], houtb[:, t, :], be2bc)

                nc.sync.dma_start(out=bc_view(yh, n0, D, NH * D), in_=houtb)

    nc.compile()
    return nc


def _get_executor(flags):
    """Build (once) and return fn(in_maps) -> list[dict] running on 8 cores."""
    if flags in _CACHE:
        return _CACHE[flags]

    _ensure_path()
    import jax
    import jax.numpy as jnp  # noqa: F401
    from jax.sharding import Mesh, PartitionSpec
    try:
        from jax.experimental.shard_map import shard_map
    except ImportError:
        from jax.shard_map import shard_map
    from concourse import bass2jax as b2j
    from concourse import mybir

    nc = _build_program(flags)
    b2j.install_neuronx_cc_hook()

    partition_name = (nc.partition_id_tensor.name
                      if nc.partition_id_tensor else None)

    in_names, out_names, out_avals, zero_shapes = [], [], [], []
    for alloc in nc.m.functions[0].allocations:
        if not isinstance(alloc, mybir.MemoryLocationSet):
            continue
        name = alloc.memorylocations[0].name
        if alloc.kind == "ExternalInput":
            if name != partition_name:
                in_names.append(name)
        elif alloc.kind == "ExternalOutput":
            shape = tuple(alloc.tensor_shape)
            dtype = mybir.dt.np(alloc.dtype)
            out_names.append(name)
            out_avals.append(jax.core.ShapedArray(shape, dtype))
            zero_shapes.append((shape, dtype))

    n_params = len(in_names)
    n_outs = len(out_names)
    all_names = list(in_names) + list(out_names)
    if partition_name is not None:
        all_names.append(partition_name)
    donate = tuple(range(n_params, n_params + n_outs))

    def _body(*args):
        operands = list(args)
        if partition_name is not None:
            operands.append(b2j.partition_id_tensor())
        outs = b2j._bass_exec_p.bind(
            *operands,
            out_avals=tuple(out_avals),
            in_names=tuple(all_names),
            out_names=tuple(out_names),
            lowering_input_output_aliases=(),
            sim_require_finite=True,
            sim_require_nnan=True,
            nc=nc,
        )
        return tuple(outs)

    devices = jax.devices()[:NCORES]
    mesh = Mesh(np.asarray(devices), ("core",))
    in_specs = (PartitionSpec("core"),) * (n_params + n_outs)
    out_specs = (PartitionSpec("core"),) * n_outs
    sharded = jax.jit(
        shard_map(_body, mesh=mesh, in_specs=in_specs, out_specs=out_specs,
                  check_rep=False),
        donate_argnums=donate, keep_unused=True)

    def run(in_maps):
        assert len(in_maps) == NCORES
        concat_in = [
            np.concatenate([np.asarray(m[name]) for m in in_maps], axis=0)
            for name in in_names
        ]
        concat_zeros = [
            np.zeros((NCORES * sh[0], *sh[1:]), dt) for sh, dt in zero_shapes
        ]
        out_arrs = sharded(*concat_in, *concat_zeros)
        out_np = [np.asarray(a) for a in out_arrs]
        return [
            {name: out_np[i].reshape(NCORES, *zero_shapes[i][0])[c]
             for i, name in enumerate(out_names)}
            for c in range(NCORES)
        ]

    _CACHE[flags] = run
    return run


def kernel(x, A_static, Wq, Wk, W1, b1, W2, b2, g1, be1, g2, be2):
    x = np.ascontiguousarray(np.asarray(x, dtype=np.float32))
    A_static = np.asarray(A_static, dtype=np.float32)
    Wq = np.asarray(Wq, dtype=np.float32)
    Wk = np.asarray(Wk, dtype=np.float32)
    W1 = np.asarray(W1, dtype=np.float32)
    b1 = np.asarray(b1, dtype=np.float32)
    W2 = np.asarray(W2, dtype=np.float32)
    b2 = np.asarray(b2, dtype=np.float32)
    g1 = np.asarray(g1, dtype=np.float32)
    be1 = np.asarray(be1, dtype=np.float32)
    g2 = np.asarray(g2, dtype=np.float32)
    be2 = np.asarray(be2, dtype=np.float32)

    g1_trivial = bool(np.all(g1 == 1.0) and np.all(be1 == 0.0))
    g2_trivial = bool(np.all(g2 == 1.0) and np.all(be2 == 0.0))
    b2_zero = bool(np.all(b2 == 0.0))
    flags = (g1_trivial, b2_zero, g2_trivial)

    # fold g1/be1 into the first FFN matmul (exact)
    if g1_trivial:
        W1f = W1
        b1f = b1
    else:
        W1f = (g1[:, None] * W1).astype(np.float32)
        b1f = (b1 + be1 @ W1).astype(np.float32)
    Wqk = np.ascontiguousarray(np.concatenate([Wq, Wk], axis=1))

    run = _get_executor(flags)

    in_maps = []
    for k in range(NCORES):
        b, h = divmod(k, 2)
        if h == 0:
            xsh = x[b]
        else:
            xsh = np.ascontiguousarray(
                np.concatenate([x[b, :, NH:, :], x[b, :, :NH, :]], axis=1))
        m = dict(xs=xsh, a_st=A_static, wqk=Wqk, w1=W1f, b1=b1f, w2=W2)
        if not b2_zero:
            m["b2"] = b2
        if not g2_trivial:
            m["g2be2"] = np.stack([g2, be2])
        if not g1_trivial:
            m["g1be1"] = np.stack([g1, be1])
        in_maps.append(m)

    results = run(in_maps)

    H_out = np.empty((B, C, N, D), dtype=np.float32)
    A = np.empty((B, C, C), dtype=np.float32)
    for k in range(NCORES):
        b, h = divmod(k, 2)
        H_out[b, :, h * NH:(h + 1) * NH, :] = results[k]["yh"]
        if h == 0:
            A[b] = results[k]["a_out"]
    return H_out, x, A
